# revision 12
# baseline (speedup 1.0000x reference)
"""Dinov3 self-attention Bass kernel for TRN2.

Sharding: data-parallel over batch. B=8 batch elements -> 8 NeuronCores,
one full attention per core, weights replicated. No collectives.

Per-core layout strategy (all matmuls bf16 x bf16 -> fp32 PSUM):
  xT  [h, s]   : x cast to bf16, DMA-transposed           (h on partitions)
  WqT/WkT/WvT/WpT [h, o] : weights cast + DMA-transposed
  qT/kT [o, s] : projections computed transposed, RoPE'd
  v    [s, o]  : projection computed natural (M = s)
  scores.T [j, i] = kT^T @ qT per head (K=d=64, two heads row-packed)
  expS.T = exp(0.125 * scores.T) on ScalarE, psum -> sbuf bf16
  PV: ctx_u.T[d, i] = (v_h | ones)^T @ expS.T  -> row 64 = softmax denominator
  normalize: ctxT = ctx_u.T * bcast(1/denom)   (DVE + DMA partition-broadcast)
  out[i, o] = ctxT^T @ WpT (+ ones x bp)       -> fp32 -> DRAM
"""

import contextlib
import os
import sys

import numpy as np

sys.path.insert(0, "/opt/trn_rl_repo")

import concourse.bacc as bacc
import concourse.bass as bass
import concourse.tile as tile
from concourse import mybir

S = 1374
H = 768
NH = 12
D = 64
NROT = 1369
PREFIX = S - NROT  # 5
B = 8

P = 128
NSTILE = (S + P - 1) // P  # 11 s-tiles, last has 94 rows
NOTILE = H // P  # 6
SPAD = NSTILE * P  # 1408
CHUNK = 687  # i-chunk (2 chunks of 687 = 1374); each spans 2 psum banks
NCHUNK = 2
BANK = 512  # fp32 elements per psum bank (matmul N limit)

F32 = mybir.dt.float32
BF16 = mybir.dt.bfloat16


def _subchunks(total):
    """Split a free-dim range into <=BANK pieces aligned to bank boundaries."""
    out = []
    off = 0
    while off < total:
        n = min(BANK, total - off)
        out.append((off, n))
        off += n
    return out


def _stile(i):
    """(start, size) of s-tile i."""
    start = i * P
    return start, min(P, S - start)


def build_kernel(nc):
    x_ext = nc.declare_dram_parameter("hidden_states", [S, H], F32, isOutput=False)
    sin_ext = nc.declare_dram_parameter("sin", [NROT, D], F32, isOutput=False)
    cos_ext = nc.declare_dram_parameter("cos", [NROT, D], F32, isOutput=False)
    wq_ext = nc.declare_dram_parameter("Wq", [H, H], F32, isOutput=False)
    bq_ext = nc.declare_dram_parameter("bq", [H], F32, isOutput=False)
    wk_ext = nc.declare_dram_parameter("Wk", [H, H], F32, isOutput=False)
    wv_ext = nc.declare_dram_parameter("Wv", [H, H], F32, isOutput=False)
    bv_ext = nc.declare_dram_parameter("bv", [H], F32, isOutput=False)
    wp_ext = nc.declare_dram_parameter("Wp", [H, H], F32, isOutput=False)
    bp_ext = nc.declare_dram_parameter("bp", [H], F32, isOutput=False)
    out_ext = nc.declare_dram_parameter("out", [S, H], F32, isOutput=True)

    with tile.TileContext(nc) as tc:
        _body(tc, x_ext, sin_ext, cos_ext, wq_ext, bq_ext, wk_ext,
              wv_ext, bv_ext, wp_ext, bp_ext, out_ext)
    nc.compile()
    return nc


def _body(tc, x_ext, sin_ext, cos_ext, wq_ext, bq_ext, wk_ext, wv_ext,
          bv_ext, wp_ext, bp_ext, out_ext):
    nc = tc.nc

    with contextlib.ExitStack() as ctx:
        # ---------------- long-lived pools ----------------
        persist = ctx.enter_context(tc.tile_pool(name="persist", bufs=1))
        psum_qk = ctx.enter_context(tc.tile_pool(name="psum_qk", bufs=2, space="PSUM"))
        psum_pv = ctx.enter_context(tc.tile_pool(name="psum_pv", bufs=2, space="PSUM"))

        xT = persist.tile([P, NOTILE, SPAD], BF16)     # xT[p, t, s] = x[s, 128t+p]
        wpT = persist.tile([P, NOTILE, H], BF16)
        qT = persist.tile([P, NOTILE, SPAD], BF16)     # roped q, [o, s] layout
        kT = persist.tile([P, NOTILE, SPAD], BF16)
        vsb = persist.tile([P, NSTILE, H], BF16)       # v[s, o] natural
        ctxT = persist.tile([P, NOTILE, SPAD], BF16)   # ctx^T [(h,d), i]
        cc2 = persist.tile([P, SPAD], BF16)            # cos^T stacked twice
        ss2 = persist.tile([P, SPAD], BF16)            # sin^T stacked, sign-baked
        bq_sb = persist.tile([P, NOTILE], F32)
        bv_row = persist.tile([1, H], BF16)
        bp_row = persist.tile([1, H], BF16)
        ones_j = persist.tile([P, 1], BF16)            # denominator column (lhsT)
        ones_row = persist.tile([1, P], BF16)          # K=1 bias matmuls (lhsT)

        nc.vector.memset(ones_j, 1.0)
        nc.vector.memset(ones_row, 1.0)

        with tc.tile_pool(name="wqkv", bufs=1) as wqkv_pool, \
             tc.tile_pool(name="ropet", bufs=2) as ropet:
            wqT = wqkv_pool.tile([P, NOTILE, H], BF16)
            wkT = wqkv_pool.tile([P, NOTILE, H], BF16)
            wvT = wqkv_pool.tile([P, NOTILE, H], BF16)

            with tc.tile_pool(name="setup_stage", bufs=2) as stage:
                # ---------------- biases ----------------
                # bq as [128, 6]: column t = bq[128t : 128t+128]
                nc.sync.dma_start(out=bq_sb,
                                  in_=bq_ext.rearrange("(t p) -> p t", p=P))
                bstage = stage.tile([1, H], F32, tag="bias_stage", bufs=1)
                nc.sync.dma_start(out=bstage,
                                  in_=bv_ext.rearrange("(a h) -> a h", a=1))
                nc.vector.tensor_copy(out=bv_row, in_=bstage)
                bstage2 = stage.tile([1, H], F32, tag="bias_stage2", bufs=1)
                nc.sync.dma_start(out=bstage2,
                                  in_=bp_ext.rearrange("(a h) -> a h", a=1))
                nc.vector.tensor_copy(out=bp_row, in_=bstage2)

                # ---------------- sin/cos tables ----------------
                # transpose [NROT, 64] f32 -> bf16 [64, s], then stack into
                # both halves of the [128, s] tables.  XBAR transpose needs
                # src cols % 128 == 0, so pad the 64 d-cols to 128.
                n_rtile = (NROT + P - 1) // P
                for src_ext, dstT in ((cos_ext, cc2), (sin_ext, ss2)):
                    csT_full = stage.tile([P, SPAD], BF16, tag="csT_full")
                    for i in range(n_rtile):
                        r0 = i * P
                        rsz = min(P, NROT - r0)
                        cst = stage.tile([P, D], F32, tag="cs_stage")
                        csb = stage.tile([P, P], BF16, tag="cs_stage_bf")
                        nc.vector.memset(csb, 0.0)
                        nc.sync.dma_start(out=cst[:rsz], in_=src_ext[r0:r0 + rsz, :])
                        nc.vector.tensor_copy(out=csb[:rsz, :D], in_=cst[:rsz])
                        nc.sync.dma_start_transpose(
                            out=csT_full[:, r0:r0 + P], in_=csb)
                    for half in range(2):
                        nc.sync.dma_start(
                            out=dstT[64 * half:64 * half + 64, :NROT],
                            in_=csT_full[:D, :NROT])
                # bake rotate_half sign into ss2: rows 0:32 and 64:96 negated
                for base in (0, 64):
                    sl = slice(base, base + 32)
                    nc.vector.tensor_scalar_mul(ss2[sl, :NROT],
                                                ss2[sl, :NROT], -1.0)

                # ---------------- load & transpose x ----------------
                for st in range(NSTILE):
                    s0, ssz = _stile(st)
                    xs = stage.tile([P, H], F32, tag="x_stage")
                    xb = stage.tile([P, H], BF16, tag="x_stage_bf")
                    if ssz < P:
                        nc.vector.memset(xb, 0.0)
                    nc.sync.dma_start(out=xs[:ssz], in_=x_ext[s0:s0 + ssz, :])
                    nc.vector.tensor_copy(out=xb[:ssz], in_=xs[:ssz])
                    for t in range(NOTILE):
                        nc.sync.dma_start_transpose(
                            out=xT[:, t, s0:s0 + P], in_=xb[:, t * P:(t + 1) * P])

                # ---------------- load & transpose weights ----------------
                for w_ext, wT in ((wq_ext, wqT), (wk_ext, wkT),
                                  (wv_ext, wvT), (wp_ext, wpT)):
                    for r in range(NOTILE):  # row tile of W (o dim)
                        ws = stage.tile([P, H], F32, tag="w_stage")
                        wb = stage.tile([P, H], BF16, tag="w_stage_bf")
                        nc.sync.dma_start(out=ws, in_=w_ext[r * P:(r + 1) * P, :])
                        nc.vector.tensor_copy(out=wb, in_=ws)
                        for t in range(NOTILE):  # h tile
                            nc.sync.dma_start_transpose(
                                out=wT[:, t, r * P:(r + 1) * P],
                                in_=wb[:, t * P:(t + 1) * P])

            # ---------------- q/k projections (transposed out) + RoPE --------
            # q^T[o, s] = sum_h WqT[h, o-tile]^T @ xT[h, s]
            for wT, dstT, bias in ((wqT, qT, True), (wkT, kT, False)):
                for ot in range(NOTILE):
                    qb = ropet.tile([P, SPAD], BF16, tag="qb")
                    for c in range(NCHUNK):
                        i0 = c * CHUNK
                        ps = psum_qk.tile([P, 2 * BANK], F32, tag="qk", name="qkps")[:, :CHUNK]
                        for kt in range(NOTILE):
                            for (o, n) in _subchunks(CHUNK):
                                nc.tensor.matmul(
                                    ps[:, o:o + n],
                                    wT[:, kt, ot * P:(ot + 1) * P],
                                    xT[:, kt, i0 + o:i0 + o + n],
                                    start=(kt == 0), stop=(kt == NOTILE - 1))
                        # evict + bias (per-partition scalar) -> bf16
                        if bias:
                            nc.vector.tensor_scalar_add(
                                qb[:, i0:i0 + CHUNK], ps, bq_sb[:, ot:ot + 1])
                        else:
                            nc.vector.tensor_copy(out=qb[:, i0:i0 + CHUNK], in_=ps)
                    # RoPE: rot[p] = qb[pair(p)] via partition-shifted DMA
                    rot = ropet.tile([P, NROT], BF16, tag="rot")
                    for (dst0, src0) in ((0, 32), (32, 0), (64, 96), (96, 64)):
                        nc.sync.dma_start(
                            out=rot[dst0:dst0 + 32, :],
                            in_=qb[src0:src0 + 32, PREFIX:PREFIX + NROT])
                    sl = slice(PREFIX, PREFIX + NROT)
                    nc.vector.tensor_mul(dstT[:, ot, sl], qb[:, sl],
                                         cc2[:, :NROT])
                    nc.vector.tensor_mul(rot, rot, ss2[:, :NROT])
                    nc.vector.tensor_add(dstT[:, ot, sl], dstT[:, ot, sl], rot)
                    nc.vector.tensor_copy(out=dstT[:, ot, 0:PREFIX],
                                          in_=qb[:, 0:PREFIX])

            # ---------------- v projection (natural out) ----------------
            for st in range(NSTILE):
                s0, ssz = _stile(st)
                ps = psum_pv.tile([P, 2 * BANK], F32, tag="pv", name="vps")[:, :H]
                for (o, n) in _subchunks(H):
                    for kt in range(NOTILE):
                        nc.tensor.matmul(
                            ps[:ssz, o:o + n],
                            xT[:, kt, s0:s0 + ssz],
                            wvT[:, kt, o:o + n],
                            start=(kt == 0), stop=False)
                    # bias: += ones[s] x bv[o]  (K=1 rank-1 update ends group)
                    nc.tensor.matmul(
                        ps[:ssz, o:o + n],
                        ones_row[:, :ssz],
                        bv_row[:, o:o + n],
                        start=False, stop=True)
                nc.vector.tensor_copy(out=vsb[:ssz, st, :], in_=ps[:ssz, :])

        # ---------------- attention (6 head pairs) ----------------
        exps_pool = ctx.enter_context(tc.tile_pool(name="exps_pool", bufs=3))
        norm_pool = ctx.enter_context(tc.tile_pool(name="norm_pool", bufs=2))
        outst = ctx.enter_context(tc.tile_pool(name="outst", bufs=2))
        rs_scratch = nc.dram_tensor("rs_scratch", [NH * NCHUNK, CHUNK], F32)

        for pt in range(NOTILE):  # head pair = heads (2pt, 2pt+1)
            for c in range(NCHUNK):
                i0 = c * CHUNK
                pv_ps = []
                for hh in range(2):
                    pv_ps.append(psum_pv.tile([P, 2 * BANK], F32, tag="pv",
                                              name=f"pvps_{pt}_{c}_{hh}")[:, :CHUNK])
                for jt in range(NSTILE):
                    j0, jsz = _stile(jt)
                    exps = []
                    for hh in range(2):  # head half: partitions 64*hh
                        hb = 64 * hh
                        sc = psum_qk.tile([P, 2 * BANK], F32, tag="qk",
                                          name=f"scps_{pt}_{c}_{jt}_{hh}")[:, :CHUNK]
                        for (o, n) in _subchunks(CHUNK):
                            nc.tensor.matmul(
                                sc[:jsz, o:o + n],
                                kT[hb:hb + 64, pt, j0:j0 + jsz],
                                qT[hb:hb + 64, pt, i0 + o:i0 + o + n],
                                start=True, stop=True)
                        es = exps_pool.tile([P, CHUNK], BF16, tag="es",
                                            name=f"es_{pt}_{c}_{jt}_{hh}")
                        nc.scalar.activation(
                            out=es[:jsz, :], in_=sc[:jsz, :],
                            func=mybir.ActivationFunctionType.Exp,
                            scale=float(D) ** -0.5)
                        exps.append(es)
                    for hh in range(2):
                        h = 2 * pt + hh
                        es = exps[hh]
                        first = (jt == 0)
                        last = (jt == NSTILE - 1)
                        for (o, n) in _subchunks(CHUNK):
                            # ctx_u^T [64, chunk]
                            nc.tensor.matmul(
                                pv_ps[hh][0:D, o:o + n],
                                vsb[:jsz, jt, h * D:(h + 1) * D],
                                es[:jsz, o:o + n],
                                start=first, stop=last)
                            # denominator row via ones column (col group 2).
                            # Shares psum banks with the ctx rows on disjoint
                            # partitions; has_written is per element so the
                            # group overlap is safe on HW.
                            nc.tensor.matmul(
                                pv_ps[hh][D:D + 1, o:o + n],
                                ones_j[:jsz, :],
                                es[:jsz, o:o + n],
                                start=first, stop=last,
                                tile_position=(0, 64),
                                skip_group_check=True)
                # normalize: ctxT = ctx_u^T * (1/denom), bcast over partitions.
                # SBUF APs can't have partition step 0, DRAM APs can -- bounce
                # the 1/denom row off a DRAM scratch row to broadcast it.
                for hh in range(2):
                    idx = (pt * NCHUNK + c) * 2 + hh
                    rsum = norm_pool.tile([1, CHUNK], F32, tag="rsum",
                                          name=f"rsum_{pt}_{c}_{hh}")
                    nc.vector.reciprocal(out=rsum, in_=pv_ps[hh][D:D + 1, :])
                    nc.sync.dma_start(out=rs_scratch[idx:idx + 1, :], in_=rsum)
                    bc = norm_pool.tile([D, CHUNK], F32, tag="bc",
                                        name=f"bc_{pt}_{c}_{hh}")
                    scr_row = rs_scratch[idx:idx + 1, :]
                    bcast_src = bass.AP(
                        tensor=scr_row.tensor, offset=scr_row.offset,
                        ap=[[0, D]] + list(scr_row.ap[1:]))
                    nc.sync.dma_start(out=bc, in_=bcast_src)
                    nc.vector.tensor_mul(
                        ctxT[64 * hh:64 * hh + 64, pt, i0:i0 + CHUNK],
                        pv_ps[hh][0:D, :], bc)

        # ---------------- output projection ----------------
        for it in range(NSTILE):
            s0, ssz = _stile(it)
            ps = psum_pv.tile([P, 2 * BANK], F32, tag="pv", name=f"ops_{it}")[:, :H]
            for (o, n) in _subchunks(H):
                for kt in range(NOTILE):
                    nc.tensor.matmul(
                        ps[:ssz, o:o + n],
                        ctxT[:, kt, s0:s0 + ssz],
                        wpT[:, kt, o:o + n],
                        start=(kt == 0), stop=False)
                nc.tensor.matmul(
                    ps[:ssz, o:o + n],
                    ones_row[:, :ssz],
                    bp_row[:, o:o + n],
                    start=False, stop=True)
            ot = outst.tile([P, H], F32, tag="ostage")
            nc.vector.tensor_copy(out=ot[:ssz], in_=ps[:ssz])
            nc.sync.dma_start(out=out_ext[s0:s0 + ssz, :], in_=ot[:ssz])


_NC_CACHE = None


def get_nc():
    global _NC_CACHE
    if _NC_CACHE is None:
        nc = bacc.Bacc(None, target_bir_lowering=False, debug=False)
        _NC_CACHE = build_kernel(nc)
    return _NC_CACHE


def kernel(**inputs):
    from concourse.bass_utils import run_bass_kernel_spmd

    nc = get_nc()
    names = ["hidden_states", "sin", "cos", "Wq", "bq", "Wk", "Wv", "bv", "Wp", "bp"]
    arrs = {k: np.ascontiguousarray(np.asarray(inputs[k], dtype=np.float32))
            for k in names}
    in_maps = []
    for b in range(B):
        m = {k: arrs[k] for k in names if k != "hidden_states"}
        m["hidden_states"] = np.ascontiguousarray(arrs["hidden_states"][b])
        in_maps.append(m)
    res = run_bass_kernel_spmd(nc, in_maps, core_ids=list(range(B)))
    out = np.stack([res.results[b]["out"] for b in range(B)], axis=0)
    return out.astype(np.float32)


if __name__ == "__main__":
    # quick smoke: build only
    nc = get_nc()
    print("built ok")


# revision 17
# speedup vs baseline: 1.2316x; 1.2316x over previous
"""Dinov3 self-attention Bass kernel for TRN2.

Sharding: data-parallel over batch. B=8 batch elements -> 8 NeuronCores,
one full attention per core, weights replicated. No collectives.

Per-core layout strategy (all matmuls bf16 x bf16 -> fp32 PSUM):
  xT  [h, s]   : x cast to bf16, DMA-transposed           (h on partitions)
  WqT/WkT/WvT/WpT [h, o] : weights cast + DMA-transposed
  qT/kT [o, s] : projections computed transposed, RoPE'd
  v    [s, o]  : projection computed natural (M = s)
  scores.T [j, i] = kT^T @ qT per head (K=d=64, two heads row-packed)
  expS.T = exp(0.125 * scores.T) on ScalarE, psum -> sbuf bf16
  PV: ctx_u.T[d, i] = (v_h | ones)^T @ expS.T  -> row 64 = softmax denominator
  normalize: ctxT = ctx_u.T * bcast(1/denom)   (DVE + DMA partition-broadcast)
  out[i, o] = ctxT^T @ WpT (+ ones x bp)       -> fp32 -> DRAM
"""

import contextlib
import os
import sys

import numpy as np

sys.path.insert(0, "/opt/trn_rl_repo")

import concourse.bacc as bacc
import concourse.bass as bass
import concourse.tile as tile
from concourse import mybir

S = 1374
H = 768
NH = 12
D = 64
NROT = 1369
PREFIX = S - NROT  # 5
B = 8

P = 128
NSTILE = (S + P - 1) // P  # 11 s-tiles, last has 94 rows
NOTILE = H // P  # 6
SPAD = NSTILE * P  # 1408
CHUNK = 687  # i-chunk (2 chunks of 687 = 1374); each spans 2 psum banks
NCHUNK = 2
BANK = 512  # fp32 elements per psum bank (matmul N limit)
SCR_W = 768  # padded width of the denominator scratch rows

F32 = mybir.dt.float32
BF16 = mybir.dt.bfloat16


def _subchunks(total):
    """Split a free-dim range into <=BANK pieces aligned to bank boundaries."""
    out = []
    off = 0
    while off < total:
        n = min(BANK, total - off)
        out.append((off, n))
        off += n
    return out


def _stile(i):
    """(start, size) of s-tile i."""
    start = i * P
    return start, min(P, S - start)


def build_kernel(nc):
    x_ext = nc.declare_dram_parameter("hidden_states", [S, H], F32, isOutput=False)
    sin_ext = nc.declare_dram_parameter("sin", [NROT, D], F32, isOutput=False)
    cos_ext = nc.declare_dram_parameter("cos", [NROT, D], F32, isOutput=False)
    wq_ext = nc.declare_dram_parameter("Wq", [H, H], F32, isOutput=False)
    bq_ext = nc.declare_dram_parameter("bq", [H], F32, isOutput=False)
    wk_ext = nc.declare_dram_parameter("Wk", [H, H], F32, isOutput=False)
    wv_ext = nc.declare_dram_parameter("Wv", [H, H], F32, isOutput=False)
    bv_ext = nc.declare_dram_parameter("bv", [H], F32, isOutput=False)
    wp_ext = nc.declare_dram_parameter("Wp", [H, H], F32, isOutput=False)
    bp_ext = nc.declare_dram_parameter("bp", [H], F32, isOutput=False)
    out_ext = nc.declare_dram_parameter("out", [S, H], F32, isOutput=True)

    with tile.TileContext(nc) as tc:
        _body(tc, x_ext, sin_ext, cos_ext, wq_ext, bq_ext, wk_ext,
              wv_ext, bv_ext, wp_ext, bp_ext, out_ext)
    nc.compile()
    return nc


def _body(tc, x_ext, sin_ext, cos_ext, wq_ext, bq_ext, wk_ext, wv_ext,
          bv_ext, wp_ext, bp_ext, out_ext):
    nc = tc.nc

    with contextlib.ExitStack() as ctx:
        # ---------------- long-lived pools ----------------
        persist = ctx.enter_context(tc.tile_pool(name="persist", bufs=1))
        psum_qk = ctx.enter_context(tc.tile_pool(name="psum_qk", bufs=2, space="PSUM"))
        psum_pv = ctx.enter_context(tc.tile_pool(name="psum_pv", bufs=2, space="PSUM"))

        xT = persist.tile([P, NOTILE, SPAD], BF16)     # xT[p, t, s] = x[s, 128t+p]
        wpT = persist.tile([P, NOTILE, H], BF16)
        qT = persist.tile([P, NOTILE, SPAD], BF16)     # roped q, [o, s] layout
        kT = persist.tile([P, NOTILE, SPAD], BF16)
        vsb = persist.tile([P, NSTILE, H], BF16)       # v[s, o] natural
        ctxT = persist.tile([P, NOTILE, SPAD], BF16)   # ctx^T [(h,d), i]
        cc2 = persist.tile([P, SPAD], BF16)            # cos^T stacked twice
        ss2 = persist.tile([P, SPAD], BF16)            # sin^T stacked, sign-baked
        bq_sb = persist.tile([P, NOTILE], F32)
        bv_row = persist.tile([1, H], BF16)
        bp_row = persist.tile([1, H], BF16)
        ones_j = persist.tile([P, 1], BF16)            # denominator column (lhsT)
        ones_row = persist.tile([1, P], BF16)          # K=1 bias matmuls (lhsT)

        nc.vector.memset(ones_j, 1.0)
        nc.vector.memset(ones_row, 1.0)

        with tc.tile_pool(name="wqkv", bufs=1) as wqkv_pool, \
             tc.tile_pool(name="ropet", bufs=2) as ropet:
            wqT = wqkv_pool.tile([P, NOTILE, H], BF16)
            wkT = wqkv_pool.tile([P, NOTILE, H], BF16)
            wvT = wqkv_pool.tile([P, NOTILE, H], BF16)

            with tc.tile_pool(name="setup_stage", bufs=2) as stage:
                # ---------------- biases ----------------
                # bq as [128, 6]: column t = bq[128t : 128t+128]
                nc.sync.dma_start(out=bq_sb,
                                  in_=bq_ext.rearrange("(t p) -> p t", p=P))
                bstage = stage.tile([1, H], F32, tag="bias_stage", bufs=1)
                nc.sync.dma_start(out=bstage,
                                  in_=bv_ext.rearrange("(a h) -> a h", a=1))
                nc.vector.tensor_copy(out=bv_row, in_=bstage)
                bstage2 = stage.tile([1, H], F32, tag="bias_stage2", bufs=1)
                nc.sync.dma_start(out=bstage2,
                                  in_=bp_ext.rearrange("(a h) -> a h", a=1))
                nc.vector.tensor_copy(out=bp_row, in_=bstage2)

                # ---------------- sin/cos tables ----------------
                # transpose [NROT, 64] f32 -> bf16 [64, s], then stack into
                # both halves of the [128, s] tables.  XBAR transpose needs
                # src cols % 128 == 0, so pad the 64 d-cols to 128.
                n_rtile = (NROT + P - 1) // P
                for src_ext, dstT in ((cos_ext, cc2), (sin_ext, ss2)):
                    csT_full = stage.tile([P, SPAD], BF16, tag="csT_full")
                    for i in range(n_rtile):
                        r0 = i * P
                        rsz = min(P, NROT - r0)
                        cst = stage.tile([P, D], F32, tag="cs_stage")
                        csb = stage.tile([P, P], BF16, tag="cs_stage_bf")
                        nc.vector.memset(csb, 0.0)
                        nc.sync.dma_start(out=cst[:rsz], in_=src_ext[r0:r0 + rsz, :])
                        nc.vector.tensor_copy(out=csb[:rsz, :D], in_=cst[:rsz])
                        nc.sync.dma_start_transpose(
                            out=csT_full[:, r0:r0 + P], in_=csb)
                    for half in range(2):
                        nc.sync.dma_start(
                            out=dstT[64 * half:64 * half + 64, :NROT],
                            in_=csT_full[:D, :NROT])
                # bake rotate_half sign into ss2: rows 0:32 and 64:96 negated
                for base in (0, 64):
                    sl = slice(base, base + 32)
                    nc.vector.tensor_scalar_mul(ss2[sl, :NROT],
                                                ss2[sl, :NROT], -1.0)

                # ---------------- load & transpose x ----------------
                for st in range(NSTILE):
                    s0, ssz = _stile(st)
                    xs = stage.tile([P, H], F32, tag="x_stage")
                    xb = stage.tile([P, H], BF16, tag="x_stage_bf")
                    if ssz < P:
                        nc.vector.memset(xb, 0.0)
                    nc.sync.dma_start(out=xs[:ssz], in_=x_ext[s0:s0 + ssz, :])
                    nc.vector.tensor_copy(out=xb[:ssz], in_=xs[:ssz])
                    nc.sync.dma_start_transpose(out=xT[:, :, s0:s0 + P], in_=xb)

                # ---------------- load & transpose weights ----------------
                for w_ext, wT in ((wq_ext, wqT), (wk_ext, wkT),
                                  (wv_ext, wvT), (wp_ext, wpT)):
                    for r in range(NOTILE):  # row tile of W (o dim)
                        ws = stage.tile([P, H], F32, tag="w_stage")
                        wb = stage.tile([P, H], BF16, tag="w_stage_bf")
                        nc.sync.dma_start(out=ws, in_=w_ext[r * P:(r + 1) * P, :])
                        nc.vector.tensor_copy(out=wb, in_=ws)
                        nc.sync.dma_start_transpose(out=wT[:, :, r * P:(r + 1) * P],
                                                    in_=wb)

            # ---------------- q/k projections (transposed out) + RoPE --------
            # q^T[o, s] = sum_h WqT[h, o-tile]^T @ xT[h, s]
            for wT, dstT, bias in ((wqT, qT, True), (wkT, kT, False)):
                for ot in range(NOTILE):
                    qb = ropet.tile([P, SPAD], BF16, tag="qb")
                    for c in range(NCHUNK):
                        i0 = c * CHUNK
                        ps = psum_qk.tile([P, 2 * BANK], F32, tag="qk", name="qkps")[:, :CHUNK]
                        for kt in range(NOTILE):
                            for (o, n) in _subchunks(CHUNK):
                                nc.tensor.matmul(
                                    ps[:, o:o + n],
                                    wT[:, kt, ot * P:(ot + 1) * P],
                                    xT[:, kt, i0 + o:i0 + o + n],
                                    start=(kt == 0), stop=(kt == NOTILE - 1))
                        # evict + bias (per-partition scalar) -> bf16 on ACT
                        # (idle during the projection phase)
                        if bias:
                            nc.scalar.add(qb[:, i0:i0 + CHUNK], ps,
                                          bq_sb[:, ot:ot + 1])
                        else:
                            nc.scalar.copy(out=qb[:, i0:i0 + CHUNK], in_=ps)
                    # RoPE: rot[p] = qb[pair(p)] via partition-shifted DMA
                    rot = ropet.tile([P, NROT], BF16, tag="rot")
                    for (dst0, src0) in ((0, 32), (32, 0), (64, 96), (96, 64)):
                        nc.sync.dma_start(
                            out=rot[dst0:dst0 + 32, :],
                            in_=qb[src0:src0 + 32, PREFIX:PREFIX + NROT])
                    sl = slice(PREFIX, PREFIX + NROT)
                    nc.vector.tensor_mul(dstT[:, ot, sl], qb[:, sl],
                                         cc2[:, :NROT])
                    nc.vector.tensor_mul(rot, rot, ss2[:, :NROT])
                    nc.vector.tensor_add(dstT[:, ot, sl], dstT[:, ot, sl], rot)
                    nc.vector.tensor_copy(out=dstT[:, ot, 0:PREFIX],
                                          in_=qb[:, 0:PREFIX])

            # ---------------- v projection (natural out) ----------------
            for st in range(NSTILE):
                s0, ssz = _stile(st)
                ps = psum_pv.tile([P, 2 * BANK], F32, tag="pv", name="vps")[:, :H]
                for (o, n) in _subchunks(H):
                    for kt in range(NOTILE):
                        nc.tensor.matmul(
                            ps[:ssz, o:o + n],
                            xT[:, kt, s0:s0 + ssz],
                            wvT[:, kt, o:o + n],
                            start=(kt == 0), stop=False)
                    # bias: += ones[s] x bv[o]  (K=1 rank-1 update ends group)
                    nc.tensor.matmul(
                        ps[:ssz, o:o + n],
                        ones_row[:, :ssz],
                        bv_row[:, o:o + n],
                        start=False, stop=True)
                nc.scalar.copy(out=vsb[:ssz, st, :], in_=ps[:ssz, :])

        # ---------------- attention (6 head pairs) ----------------
        exps_pool = ctx.enter_context(tc.tile_pool(name="exps_pool", bufs=3))
        norm_pool = ctx.enter_context(tc.tile_pool(name="norm_pool", bufs=2))
        outst = ctx.enter_context(tc.tile_pool(name="outst", bufs=2))
        dram_pool = ctx.enter_context(
            tc.tile_pool(name="dram_pool", bufs=1, space="DRAM"))
        rs_scratch = dram_pool.tile([NH * NCHUNK, SCR_W], F32)
        # prefill with 1.0 so the 687:768 pad cols stay finite under recip
        ones_f32 = norm_pool.tile([1, SCR_W], F32, tag="ones_f32", bufs=1)
        nc.vector.memset(ones_f32, 1.0)
        for idx in range(NH * NCHUNK):
            nc.sync.dma_start(out=rs_scratch[idx:idx + 1, :], in_=ones_f32)

        for pt in range(NOTILE):  # head pair = heads (2pt, 2pt+1)
            for c in range(NCHUNK):
                i0 = c * CHUNK
                pv_ps = []
                for hh in range(2):
                    pv_ps.append(psum_pv.tile([P, 2 * BANK], F32, tag="pv",
                                              name=f"pvps_{pt}_{c}_{hh}")[:, :CHUNK])
                for jt in range(NSTILE):
                    j0, jsz = _stile(jt)
                    exps = []
                    for hh in range(2):  # head half: partitions 64*hh
                        hb = 64 * hh
                        sc = psum_qk.tile([P, 2 * BANK], F32, tag="qk",
                                          name=f"scps_{pt}_{c}_{jt}_{hh}")[:, :CHUNK]
                        for (o, n) in _subchunks(CHUNK):
                            nc.tensor.matmul(
                                sc[:jsz, o:o + n],
                                kT[hb:hb + 64, pt, j0:j0 + jsz],
                                qT[hb:hb + 64, pt, i0 + o:i0 + o + n],
                                start=True, stop=True)
                        es = exps_pool.tile([P, CHUNK], BF16, tag="es",
                                            name=f"es_{pt}_{c}_{jt}_{hh}")
                        nc.scalar.activation(
                            out=es[:jsz, :], in_=sc[:jsz, :],
                            func=mybir.ActivationFunctionType.Exp,
                            scale=float(D) ** -0.5)
                        exps.append(es)
                    for hh in range(2):
                        h = 2 * pt + hh
                        es = exps[hh]
                        first = (jt == 0)
                        last = (jt == NSTILE - 1)
                        for (o, n) in _subchunks(CHUNK):
                            # ctx_u^T [64, chunk]
                            nc.tensor.matmul(
                                pv_ps[hh][0:D, o:o + n],
                                vsb[:jsz, jt, h * D:(h + 1) * D],
                                es[:jsz, o:o + n],
                                start=first, stop=last)
                            # denominator row via ones column (col group 2).
                            # Shares psum banks with the ctx rows on disjoint
                            # partitions; has_written is per element so the
                            # group overlap is safe on HW.
                            nc.tensor.matmul(
                                pv_ps[hh][D:D + 1, o:o + n],
                                ones_j[:jsz, :],
                                es[:jsz, o:o + n],
                                start=first, stop=last,
                                tile_position=(0, 64),
                                skip_group_check=True)
                # normalize: ctxT = ctx_u^T * (1/denom), bcast over partitions.
                # 1. evict psum to sbuf immediately (frees the pv psum slot).
                # 2. reciprocal is free-dim-serial (~8 cyc/elem), so reshape
                #    the 687-wide denom row into [128, 6] via a DRAM bounce
                #    and run the recip across partitions instead.
                # 3. SBUF APs can't have partition step 0, DRAM APs can --
                #    broadcast-read the recip'd row from the DRAM scratch.
                for hh in range(2):
                    idx = (pt * NCHUNK + c) * 2 + hh
                    stg = norm_pool.tile([D + 1, CHUNK], F32, tag="stg",
                                         name=f"stg_{pt}_{c}_{hh}")
                    nc.vector.tensor_copy(out=stg, in_=pv_ps[hh][0:D + 1, :])
                    nc.sync.dma_start(out=rs_scratch[idx:idx + 1, :CHUNK],
                                      in_=stg[D:D + 1, :])
                    rsh = norm_pool.tile([P, SCR_W // P], F32, tag="rsh",
                                         name=f"rsh_{pt}_{c}_{hh}")
                    nc.sync.dma_start(
                        out=rsh, in_=rs_scratch[idx, :].rearrange(
                            "(i p) -> p i", p=P))
                    nc.vector.reciprocal(out=rsh, in_=rsh)
                    nc.sync.dma_start(
                        out=rs_scratch[idx, :].rearrange("(i p) -> p i", p=P),
                        in_=rsh)
                    bc = norm_pool.tile([D, CHUNK], F32, tag="bc",
                                        name=f"bc_{pt}_{c}_{hh}")
                    scr_row = rs_scratch[idx:idx + 1, :CHUNK]
                    bcast_src = bass.AP(
                        tensor=scr_row.tensor, offset=scr_row.offset,
                        ap=[[0, D]] + list(scr_row.ap[1:]))
                    nc.sync.dma_start(out=bc, in_=bcast_src)
                    nc.vector.tensor_mul(
                        ctxT[64 * hh:64 * hh + 64, pt, i0:i0 + CHUNK],
                        stg[0:D, :], bc)

        # ---------------- output projection ----------------
        for it in range(NSTILE):
            s0, ssz = _stile(it)
            ps = psum_pv.tile([P, 2 * BANK], F32, tag="pv", name=f"ops_{it}")[:, :H]
            for (o, n) in _subchunks(H):
                for kt in range(NOTILE):
                    nc.tensor.matmul(
                        ps[:ssz, o:o + n],
                        ctxT[:, kt, s0:s0 + ssz],
                        wpT[:, kt, o:o + n],
                        start=(kt == 0), stop=False)
                nc.tensor.matmul(
                    ps[:ssz, o:o + n],
                    ones_row[:, :ssz],
                    bp_row[:, o:o + n],
                    start=False, stop=True)
            ot = outst.tile([P, H], F32, tag="ostage")
            nc.scalar.copy(out=ot[:ssz], in_=ps[:ssz])
            nc.sync.dma_start(out=out_ext[s0:s0 + ssz, :], in_=ot[:ssz])


_NC_CACHE = None


def get_nc():
    global _NC_CACHE
    if _NC_CACHE is None:
        nc = bacc.Bacc(None, target_bir_lowering=False, debug=False)
        _NC_CACHE = build_kernel(nc)
    return _NC_CACHE


def kernel(**inputs):
    from concourse.bass_utils import run_bass_kernel_spmd

    nc = get_nc()
    names = ["hidden_states", "sin", "cos", "Wq", "bq", "Wk", "Wv", "bv", "Wp", "bp"]
    arrs = {k: np.ascontiguousarray(np.asarray(inputs[k], dtype=np.float32))
            for k in names}
    in_maps = []
    for b in range(B):
        m = {k: arrs[k] for k in names if k != "hidden_states"}
        m["hidden_states"] = np.ascontiguousarray(arrs["hidden_states"][b])
        in_maps.append(m)
    res = run_bass_kernel_spmd(nc, in_maps, core_ids=list(range(B)))
    out = np.stack([res.results[b]["out"] for b in range(B)], axis=0)
    return out.astype(np.float32)


if __name__ == "__main__":
    # quick smoke: build only
    nc = get_nc()
    print("built ok")


# revision 19
# speedup vs baseline: 2.0094x; 1.6316x over previous
"""Dinov3 self-attention Bass kernel for TRN2.

Sharding: data-parallel over batch. B=8 batch elements -> 8 NeuronCores,
one full attention per core, weights replicated. No collectives.

Per-core layout strategy (all matmuls bf16 x bf16 -> fp32 PSUM):
  xT  [h, s]   : x cast to bf16, DMA-transposed           (h on partitions)
  WqT/WkT/WvT/WpT [h, o] : weights cast + DMA-transposed
  qT/kT [o, s] : projections computed transposed, RoPE'd
  v    [s, o]  : projection computed natural (M = s)
  scores.T [j, i] = kT^T @ qT per head (K=d=64, two heads row-packed)
  expS.T = exp(0.125 * scores.T) on ScalarE, psum -> sbuf bf16
  PV: ctx_u.T[d, i] = (v_h | ones)^T @ expS.T  -> row 64 = softmax denominator
  normalize: ctxT = ctx_u.T * bcast(1/denom)   (DVE + DMA partition-broadcast)
  out[i, o] = ctxT^T @ WpT (+ ones x bp)       -> fp32 -> DRAM
"""

import contextlib
import os
import sys

import numpy as np

sys.path.insert(0, "/opt/trn_rl_repo")

import concourse.bacc as bacc
import concourse.bass as bass
import concourse.tile as tile
from concourse import mybir

S = 1374
H = 768
NH = 12
D = 64
NROT = 1369
PREFIX = S - NROT  # 5
B = 8

P = 128
NSTILE = (S + P - 1) // P  # 11 s-tiles, last has 94 rows
NOTILE = H // P  # 6
SPAD = NSTILE * P  # 1408
CHUNK = 687  # i-chunk (2 chunks of 687 = 1374); each spans 2 psum banks
NCHUNK = 2
BANK = 512  # fp32 elements per psum bank (matmul N limit)
SCR_W = 768  # padded width of the denominator scratch rows

F32 = mybir.dt.float32
BF16 = mybir.dt.bfloat16


def _subchunks(total):
    """Split a free-dim range into <=BANK pieces aligned to bank boundaries."""
    out = []
    off = 0
    while off < total:
        n = min(BANK, total - off)
        out.append((off, n))
        off += n
    return out


def _stile(i):
    """(start, size) of s-tile i."""
    start = i * P
    return start, min(P, S - start)


def build_kernel(nc):
    x_ext = nc.declare_dram_parameter("hidden_states", [S, H], F32, isOutput=False)
    sin_ext = nc.declare_dram_parameter("sin", [NROT, D], F32, isOutput=False)
    cos_ext = nc.declare_dram_parameter("cos", [NROT, D], F32, isOutput=False)
    wq_ext = nc.declare_dram_parameter("Wq", [H, H], F32, isOutput=False)
    bq_ext = nc.declare_dram_parameter("bq", [H], F32, isOutput=False)
    wk_ext = nc.declare_dram_parameter("Wk", [H, H], F32, isOutput=False)
    wv_ext = nc.declare_dram_parameter("Wv", [H, H], F32, isOutput=False)
    bv_ext = nc.declare_dram_parameter("bv", [H], F32, isOutput=False)
    wp_ext = nc.declare_dram_parameter("Wp", [H, H], F32, isOutput=False)
    bp_ext = nc.declare_dram_parameter("bp", [H], F32, isOutput=False)
    out_ext = nc.declare_dram_parameter("out", [S, H], F32, isOutput=True)

    with tile.TileContext(nc) as tc:
        _body(tc, x_ext, sin_ext, cos_ext, wq_ext, bq_ext, wk_ext,
              wv_ext, bv_ext, wp_ext, bp_ext, out_ext)
    nc.compile()
    return nc


def _body(tc, x_ext, sin_ext, cos_ext, wq_ext, bq_ext, wk_ext, wv_ext,
          bv_ext, wp_ext, bp_ext, out_ext):
    nc = tc.nc
    from concourse.masks import make_identity

    with contextlib.ExitStack() as ctx:
        # ---------------- long-lived pools ----------------
        persist = ctx.enter_context(tc.tile_pool(name="persist", bufs=1))
        psum_qk = ctx.enter_context(tc.tile_pool(name="psum_qk", bufs=2, space="PSUM"))
        psum_pv = ctx.enter_context(tc.tile_pool(name="psum_pv", bufs=2, space="PSUM"))

        xT = persist.tile([P, NOTILE, SPAD], BF16)     # xT[p, t, s] = x[s, 128t+p]
        wpT = persist.tile([P, NOTILE, H], BF16)
        qT = persist.tile([P, NOTILE, SPAD], BF16)     # roped q, [o, s] layout
        kT = persist.tile([P, NOTILE, SPAD], BF16)
        # v[s, (h, d|1)]: per head 64 v columns + a ones column, so the PV
        # matmul computes ctx rows AND the softmax denominator in one M=65 MM
        vsb = persist.tile([P, NSTILE, NH, D + 1], BF16)
        ctxT = persist.tile([P, NOTILE, SPAD], BF16)   # ctx^T [(h,d), i]
        cc2 = persist.tile([P, SPAD], BF16)            # cos^T stacked twice
        ss2 = persist.tile([P, SPAD], BF16)            # sin^T stacked, sign-baked
        bq_sb = persist.tile([P, NOTILE], F32)
        bv_row = persist.tile([1, H], BF16)
        bp_row = persist.tile([1, H], BF16)
        ones_row = persist.tile([1, P], BF16)          # K=1 bias matmuls (lhsT)
        ident = persist.tile([P, P], BF16)             # PE-transpose identity

        nc.vector.memset(ones_row, 1.0)
        make_identity(nc, ident)
        # ones columns of vsb (index 64 of each head's slot)
        nc.vector.memset(vsb[:, :, :, D:D + 1], 1.0)

        with tc.tile_pool(name="wqkv", bufs=1) as wqkv_pool, \
             tc.tile_pool(name="ropet", bufs=3) as ropet, \
             tc.tile_pool(name="setup_stage", bufs=2) as stage:
            wqT = wqkv_pool.tile([P, NOTILE, H], BF16)
            wkT = wqkv_pool.tile([P, NOTILE, H], BF16)
            wvT = wqkv_pool.tile([P, NOTILE, H], BF16)

            # ---------------- biases ----------------
            # bq as [128, 6]: column t = bq[128t : 128t+128]
            nc.sync.dma_start(out=bq_sb,
                              in_=bq_ext.rearrange("(t p) -> p t", p=P))
            bstage = stage.tile([1, H], F32, tag="bias_stage", bufs=1)
            nc.sync.dma_start(out=bstage,
                              in_=bv_ext.rearrange("(a h) -> a h", a=1))
            nc.vector.tensor_copy(out=bv_row, in_=bstage)
            bstage2 = stage.tile([1, H], F32, tag="bias_stage2", bufs=1)
            nc.sync.dma_start(out=bstage2,
                              in_=bp_ext.rearrange("(a h) -> a h", a=1))
            nc.vector.tensor_copy(out=bp_row, in_=bstage2)

            # ---------------- sin/cos tables (PE transpose) ----------------
            # [NROT, 64] f32 -> bf16, transpose 128-row tiles on the PE
            # (keeps the DMA queue free), stack into both halves of [128, s]
            n_rtile = (NROT + P - 1) // P
            for src_ext, dstT in ((cos_ext, cc2), (sin_ext, ss2)):
                for i in range(n_rtile):
                    r0 = i * P
                    rsz = min(P, NROT - r0)
                    cst = stage.tile([P, D], F32, tag="cs_stage")
                    csb = stage.tile([P, P], BF16, tag="cs_stage_bf")
                    nc.sync.dma_start(out=cst[:rsz], in_=src_ext[r0:r0 + rsz, :])
                    nc.vector.tensor_copy(out=csb[:rsz, :D], in_=cst[:rsz])
                    tp = psum_qk.tile([P, P], BF16, tag="qk", name=f"cstp_{i}")
                    nc.tensor.transpose(tp[:, :rsz], csb[:rsz, :], ident[:rsz, :rsz])
                    for half in range(2):
                        nc.scalar.copy(out=dstT[64 * half:64 * half + 64,
                                                r0:r0 + rsz],
                                       in_=tp[:D, :rsz])
            # bake rotate_half sign into ss2: rows 0:32 and 64:96 negated
            for base in (0, 64):
                sl = slice(base, base + 32)
                nc.vector.tensor_scalar_mul(ss2[sl, :NROT],
                                            ss2[sl, :NROT], -1.0)

            def load_weight(w_ext, wT):
                for r in range(NOTILE):  # row tile of W (o dim)
                    ws = stage.tile([P, H], F32, tag="w_stage", name=f"ws_{r}")
                    wb = stage.tile([P, H], BF16, tag="w_stage_bf", name=f"wb_{r}")
                    nc.sync.dma_start(out=ws, in_=w_ext[r * P:(r + 1) * P, :])
                    nc.vector.tensor_copy(out=wb, in_=ws)
                    nc.sync.dma_start_transpose(out=wT[:, :, r * P:(r + 1) * P],
                                                in_=wb)

            def qk_proj(wT, dstT, bias):
                for ot in range(NOTILE):
                    qb = ropet.tile([P, SPAD], BF16, tag="qb", name=f"qb_{ot}")
                    for c in range(NCHUNK):
                        i0 = c * CHUNK
                        ps = psum_qk.tile([P, 2 * BANK], F32, tag="qk",
                                          name="qkps")[:, :CHUNK]
                        for kt in range(NOTILE):
                            for (o, n) in _subchunks(CHUNK):
                                nc.tensor.matmul(
                                    ps[:, o:o + n],
                                    wT[:, kt, ot * P:(ot + 1) * P],
                                    xT[:, kt, i0 + o:i0 + o + n],
                                    start=(kt == 0), stop=(kt == NOTILE - 1))
                        # evict + bias (per-partition scalar) -> bf16 on ACT
                        # (idle during the projection phase)
                        if bias:
                            nc.scalar.add(qb[:, i0:i0 + CHUNK], ps,
                                          bq_sb[:, ot:ot + 1])
                        else:
                            nc.scalar.copy(out=qb[:, i0:i0 + CHUNK], in_=ps)
                    # RoPE: rot[p] = qb[pair(p)] via partition-shifted DMA
                    rot = ropet.tile([P, NROT], BF16, tag="rot", name=f"rot_{ot}")
                    for (dst0, src0) in ((0, 32), (32, 0), (64, 96), (96, 64)):
                        nc.sync.dma_start(
                            out=rot[dst0:dst0 + 32, :],
                            in_=qb[src0:src0 + 32, PREFIX:PREFIX + NROT])
                    sl = slice(PREFIX, PREFIX + NROT)
                    nc.vector.tensor_mul(dstT[:, ot, sl], qb[:, sl],
                                         cc2[:, :NROT])
                    nc.vector.tensor_mul(rot, rot, ss2[:, :NROT])
                    nc.vector.tensor_add(dstT[:, ot, sl], dstT[:, ot, sl], rot)
                    nc.vector.tensor_copy(out=dstT[:, ot, 0:PREFIX],
                                          in_=qb[:, 0:PREFIX])

            # emission order = DMA queue order: Wq, x, (q-proj), Wk, (k-proj),
            # Wv, (v-proj), Wp -- gets the PE going as early as possible
            load_weight(wq_ext, wqT)

            # ---------------- load & transpose x ----------------
            for st in range(NSTILE):
                s0, ssz = _stile(st)
                xs = stage.tile([P, H], F32, tag="x_stage", name=f"xs_{st}")
                xb = stage.tile([P, H], BF16, tag="x_stage_bf", name=f"xb_{st}")
                if ssz < P:
                    nc.vector.memset(xb, 0.0)
                nc.sync.dma_start(out=xs[:ssz], in_=x_ext[s0:s0 + ssz, :])
                nc.vector.tensor_copy(out=xb[:ssz], in_=xs[:ssz])
                nc.sync.dma_start_transpose(out=xT[:, :, s0:s0 + P], in_=xb)

            qk_proj(wqT, qT, True)
            load_weight(wk_ext, wkT)
            qk_proj(wkT, kT, False)
            load_weight(wv_ext, wvT)

            # ---------------- v projection (natural out) ----------------
            for st in range(NSTILE):
                s0, ssz = _stile(st)
                ps = psum_pv.tile([P, 2 * BANK], F32, tag="pv", name="vps")[:, :H]
                for (o, n) in _subchunks(H):
                    for kt in range(NOTILE):
                        nc.tensor.matmul(
                            ps[:ssz, o:o + n],
                            xT[:, kt, s0:s0 + ssz],
                            wvT[:, kt, o:o + n],
                            start=(kt == 0), stop=False)
                    # bias: += ones[s] x bv[o]  (K=1 rank-1 update ends group)
                    nc.tensor.matmul(
                        ps[:ssz, o:o + n],
                        ones_row[:, :ssz],
                        bv_row[:, o:o + n],
                        start=False, stop=True)
                # scatter the 12 heads into their 65-wide slots
                nc.scalar.copy(
                    out=vsb[:ssz, st, :, 0:D],
                    in_=ps[:ssz, :].rearrange("p (h d) -> p h d", h=NH))

            load_weight(wp_ext, wpT)

        # ---------------- attention (6 head pairs) ----------------
        exps_pool = ctx.enter_context(tc.tile_pool(name="exps_pool", bufs=4))
        norm_pool = ctx.enter_context(tc.tile_pool(name="norm_pool", bufs=2))
        outst = ctx.enter_context(tc.tile_pool(name="outst", bufs=2))
        dram_pool = ctx.enter_context(
            tc.tile_pool(name="dram_pool", bufs=1, space="DRAM"))
        rs_scratch = dram_pool.tile([NH * NCHUNK, SCR_W], F32)
        # prefill with 1.0 so the 687:768 pad cols stay finite under recip
        ones_f32 = norm_pool.tile([1, SCR_W], F32, tag="ones_f32", bufs=1)
        nc.vector.memset(ones_f32, 1.0)
        for idx in range(NH * NCHUNK):
            nc.sync.dma_start(out=rs_scratch[idx:idx + 1, :], in_=ones_f32)

        for pt in range(NOTILE):  # head pair = heads (2pt, 2pt+1)
            for c in range(NCHUNK):
                i0 = c * CHUNK
                pv_ps = []
                for hh in range(2):
                    pv_ps.append(psum_pv.tile([P, 2 * BANK], F32, tag="pv",
                                              name=f"pvps_{pt}_{c}_{hh}")[:, :CHUNK])
                for jt in range(NSTILE):
                    j0, jsz = _stile(jt)
                    exps = []
                    for hh in range(2):  # head half: partitions 64*hh
                        hb = 64 * hh
                        sc = psum_qk.tile([P, 2 * BANK], F32, tag="qk",
                                          name=f"scps_{pt}_{c}_{jt}_{hh}")[:, :CHUNK]
                        for (o, n) in _subchunks(CHUNK):
                            nc.tensor.matmul(
                                sc[:jsz, o:o + n],
                                kT[hb:hb + 64, pt, j0:j0 + jsz],
                                qT[hb:hb + 64, pt, i0 + o:i0 + o + n],
                                start=True, stop=True)
                        es = exps_pool.tile([P, CHUNK], BF16, tag="es",
                                            name=f"es_{pt}_{c}_{jt}_{hh}")
                        nc.scalar.activation(
                            out=es[:jsz, :], in_=sc[:jsz, :],
                            func=mybir.ActivationFunctionType.Exp,
                            scale=float(D) ** -0.5)
                        exps.append(es)
                    for hh in range(2):
                        h = 2 * pt + hh
                        es = exps[hh]
                        first = (jt == 0)
                        last = (jt == NSTILE - 1)
                        for (o, n) in _subchunks(CHUNK):
                            # ctx_u^T rows 0:64 + denominator row 64, one MM
                            nc.tensor.matmul(
                                pv_ps[hh][0:D + 1, o:o + n],
                                vsb[:jsz, jt, h, :],
                                es[:jsz, o:o + n],
                                start=first, stop=last)
                # normalize: ctxT = ctx_u^T * (1/denom), bcast over partitions.
                # 1. evict psum to sbuf immediately (frees the pv psum slot).
                # 2. reciprocal is free-dim-serial (~8 cyc/elem), so reshape
                #    the 687-wide denom row into [128, 6] via a DRAM bounce
                #    and run the recip across partitions instead.
                # 3. SBUF APs can't have partition step 0, DRAM APs can --
                #    broadcast-read the recip'd row from the DRAM scratch.
                for hh in range(2):
                    idx = (pt * NCHUNK + c) * 2 + hh
                    stg = norm_pool.tile([D + 1, CHUNK], F32, tag="stg",
                                         name=f"stg_{pt}_{c}_{hh}")
                    nc.vector.tensor_copy(out=stg, in_=pv_ps[hh][0:D + 1, :])
                    nc.sync.dma_start(out=rs_scratch[idx:idx + 1, :CHUNK],
                                      in_=stg[D:D + 1, :])
                    rsh = norm_pool.tile([P, SCR_W // P], F32, tag="rsh",
                                         name=f"rsh_{pt}_{c}_{hh}")
                    nc.sync.dma_start(
                        out=rsh, in_=rs_scratch[idx, :].rearrange(
                            "(i p) -> p i", p=P))
                    nc.vector.reciprocal(out=rsh, in_=rsh)
                    nc.sync.dma_start(
                        out=rs_scratch[idx, :].rearrange("(i p) -> p i", p=P),
                        in_=rsh)
                    bc = norm_pool.tile([D, CHUNK], F32, tag="bc",
                                        name=f"bc_{pt}_{c}_{hh}")
                    scr_row = rs_scratch[idx:idx + 1, :CHUNK]
                    bcast_src = bass.AP(
                        tensor=scr_row.tensor, offset=scr_row.offset,
                        ap=[[0, D]] + list(scr_row.ap[1:]))
                    nc.sync.dma_start(out=bc, in_=bcast_src)
                    nc.vector.tensor_mul(
                        ctxT[64 * hh:64 * hh + 64, pt, i0:i0 + CHUNK],
                        stg[0:D, :], bc)

        # ---------------- output projection ----------------
        for it in range(NSTILE):
            s0, ssz = _stile(it)
            ps = psum_pv.tile([P, 2 * BANK], F32, tag="pv", name=f"ops_{it}")[:, :H]
            for (o, n) in _subchunks(H):
                for kt in range(NOTILE):
                    nc.tensor.matmul(
                        ps[:ssz, o:o + n],
                        ctxT[:, kt, s0:s0 + ssz],
                        wpT[:, kt, o:o + n],
                        start=(kt == 0), stop=False)
                nc.tensor.matmul(
                    ps[:ssz, o:o + n],
                    ones_row[:, :ssz],
                    bp_row[:, o:o + n],
                    start=False, stop=True)
            ot = outst.tile([P, H], F32, tag="ostage", name=f"ost_{it}")
            nc.scalar.copy(out=ot[:ssz], in_=ps[:ssz])
            nc.sync.dma_start(out=out_ext[s0:s0 + ssz, :], in_=ot[:ssz])


_NC_CACHE = None


def get_nc():
    global _NC_CACHE
    if _NC_CACHE is None:
        nc = bacc.Bacc(None, target_bir_lowering=False, debug=False)
        _NC_CACHE = build_kernel(nc)
    return _NC_CACHE


def kernel(**inputs):
    from concourse.bass_utils import run_bass_kernel_spmd

    nc = get_nc()
    names = ["hidden_states", "sin", "cos", "Wq", "bq", "Wk", "Wv", "bv", "Wp", "bp"]
    arrs = {k: np.ascontiguousarray(np.asarray(inputs[k], dtype=np.float32))
            for k in names}
    in_maps = []
    for b in range(B):
        m = {k: arrs[k] for k in names if k != "hidden_states"}
        m["hidden_states"] = np.ascontiguousarray(arrs["hidden_states"][b])
        in_maps.append(m)
    res = run_bass_kernel_spmd(nc, in_maps, core_ids=list(range(B)))
    out = np.stack([res.results[b]["out"] for b in range(B)], axis=0)
    return out.astype(np.float32)


if __name__ == "__main__":
    # quick smoke: build only
    nc = get_nc()
    print("built ok")
